# revision 30
# baseline (speedup 1.0000x reference)
"""Multi-head causal attention (B=4, S=2048, D=1024, H=16, HD=64) on 8 trn2 cores.

Sharding: core c = (batch b = c//2, head-group hg = c%2, 8 heads each).
Each core computes Q/K/V projections restricted to its head-group, causal
attention for those heads, and a partial output projection (contraction over
its 512 ctx features). Host sums the two partial outputs per batch.

Per-core kernel layout choices:
  - QT/KT stored feature-major [head_dim, tokens] with head pairs stacked on
    partitions (0-63 / 64-127) so score matmuls for a pair run concurrently
    via PE row tiling.
  - Scores are computed transposed (ST[k, q] = K_h @ Q_h^T) so the exp'd P
    tiles are already key-major, exactly what the ctx matmul (lhsT=V) needs.
  - Softmax denominators come free out of the ctx matmul via one-hot columns
    appended to V (even heads: feat rows 0..63, denom row 64+pair; odd heads:
    feat rows 64..127, denom row pair — every evacuation stays on its own
    partitions, no cross-partition moves anywhere).
  - Normalization: reciprocal of denominators, broadcast across partitions
    with a tiny selector matmul, then one multiply per head pair.
"""

from contextlib import ExitStack

import numpy as np
import ml_dtypes

import concourse.tile as tile
from concourse import bacc, mybir
from concourse.bass_utils import run_bass_kernel_spmd

BF16 = mybir.dt.bfloat16
F32 = mybir.dt.float32

B, S, D = 4, 2048, 1024
HG = 8            # heads per core
HD = 64           # head dim
DH = HG * HD      # 512 features per core
P = 128
NKT = D // P      # 8 contraction tiles over model dim
NTT = S // P      # 16 token tiles of 128
NQC = S // 512    # 4 query chunks of 512
ESTR = 74         # even-head V block stride: 64 feat + 8 one-hot + 2 pad
OBASE = 4 * ESTR  # 296: start of odd-head V blocks (128 cols each)
VROW = OBASE + 4 * 128  # 808 cols per key-tile row of V_sb

_NC_CACHE = {}


def _build_nc():
    nc = bacc.Bacc("TRN2", target_bir_lowering=False, debug=False)

    xT = nc.dram_tensor("xT", [D, S], BF16, kind="ExternalInput")
    wqT = nc.dram_tensor("wqT", [D, DH], BF16, kind="ExternalInput")
    wkT = nc.dram_tensor("wkT", [D, DH], BF16, kind="ExternalInput")
    wvT = nc.dram_tensor("wvT", [D, DH], BF16, kind="ExternalInput")
    woT = nc.dram_tensor("woT", [DH, D], BF16, kind="ExternalInput")
    tri = nc.dram_tensor("tri", [P, P], BF16, kind="ExternalInput")
    sel = nc.dram_tensor("sel", [P, 2 * 4 * HD], F32, kind="ExternalInput")
    out = nc.dram_tensor("out", [S, D], F32, kind="ExternalOutput")

    with tile.TileContext(nc) as tc, ExitStack() as ctx:
        _emit(ctx, tc, nc, xT, wqT, wkT, wvT, woT, tri, sel, out)
    nc.compile()
    return nc


def _emit(ctx, tc, nc, xT, wqT, wkT, wvT, woT, tri, sel, out):
    Exp = mybir.ActivationFunctionType.Exp

    sb = ctx.enter_context(tc.tile_pool(name="sb", bufs=1))
    p_pool = ctx.enter_context(tc.tile_pool(name="p", bufs=6))
    bc_pool = ctx.enter_context(tc.tile_pool(name="bc", bufs=2))
    o_pool = ctx.enter_context(tc.tile_pool(name="o", bufs=2))
    ps_s = ctx.enter_context(tc.tile_pool(name="ps_s", bufs=2, space="PSUM"))
    ps_c = ctx.enter_context(tc.tile_pool(name="ps_c", bufs=2, space="PSUM"))
    ps_m = ctx.enter_context(tc.tile_pool(name="ps_m", bufs=2, space="PSUM"))

    # ---- persistent SBUF tensors ----
    xT_sb = sb.tile([P, NKT, S], BF16)
    wq_sb = sb.tile([P, NKT, DH], BF16)
    wk_sb = sb.tile([P, NKT, DH], BF16)
    wv_sb = sb.tile([P, NKT, DH], BF16)
    wo_sb = sb.tile([P, DH // P, D], BF16)
    tri_sb = sb.tile([P, P], BF16)
    qt_sb = sb.tile([P, 4, S], BF16)      # [hd (2 heads), pair, tokens]
    kt_sb = sb.tile([P, 4, S], BF16)
    v_sb = sb.tile([P, NTT, VROW], BF16)
    uctx_sb = sb.tile([P, 4, S], BF16)    # unnormalized ctxT, pair-stacked
    ctxT_sb = sb.tile([P, 4, S], BF16)    # normalized ctxT, pair-stacked
    den_sb = sb.tile([P, S], BF16)        # rows 0-3 (odd heads), 64-67 (even)
    rec_sb = sb.tile([P, S], F32)
    scr_sb = sb.tile([P, 512], F32)       # ln(den) scratch for the ACT recip
    sel_sb = sb.tile([P, 2 * 4 * HD], F32)

    # DMA order: what the first projection needs comes first
    xT_r = xT.rearrange("(t p) n -> p t n", p=P)
    wq_r = wqT.rearrange("(t p) n -> p t n", p=P)
    nc.sync.dma_start(xT_sb[:, :, 0:512], xT_r[:, :, 0:512])
    for ft in range(4):
        # per-feature-tile chunks so Q-proj(ft=0) starts as early as possible
        nc.gpsimd.dma_start(
            wq_sb[:, :, ft * P : (ft + 1) * P], wq_r[:, :, ft * P : (ft + 1) * P]
        )
    nc.gpsimd.dma_start(wk_sb[:], wkT.rearrange("(t p) n -> p t n", p=P))
    nc.sync.dma_start(wv_sb[:], wvT.rearrange("(t p) n -> p t n", p=P))
    nc.sync.dma_start(tri_sb[:], tri[:])
    for qc in range(1, NQC):
        nc.sync.dma_start(
            xT_sb[:, :, qc * 512 : (qc + 1) * 512], xT_r[:, :, qc * 512 : (qc + 1) * 512]
        )
    nc.gpsimd.dma_start(wo_sb[:], woT.rearrange("(t p) n -> p t n", p=P))
    nc.gpsimd.dma_start(sel_sb[:], sel[:])

    # ---- constant patterns: V one-hot columns ----
    nc.vector.memset(v_sb[:], 0.0)
    # rec rows read by the selector matmul before first written: keep finite
    nc.vector.memset(rec_sb[64:68, :], 1.0)
    nc.vector.memset(rec_sb[0:4, :], 1.0)
    for j in range(4):
        # even head 2j: one at col 74j + 64 + j  (-> denom row 64+j)
        nc.vector.memset(v_sb[:, :, 75 * j + 64 : 75 * j + 65], 1.0)
        # odd head 2j+1: one at col 296 + 128j + j  (-> denom row j)
        nc.vector.memset(v_sb[:, :, OBASE + 129 * j : OBASE + 129 * j + 1], 1.0)

    Ln = mybir.ActivationFunctionType.Ln

    def emit_recip(qc, rows):
        # 1/d = exp(-ln d) on ACT: ~6x lower latency than DVE reciprocal
        qs = slice(qc * 512, (qc + 1) * 512)
        for lo in (64, 0):
            r = slice(lo, lo + rows)
            nc.scalar.activation(scr_sb[r, 0:512], den_sb[r, qs], Ln)
            nc.scalar.activation(rec_sb[r, qs], scr_sb[r, 0:512], Exp, scale=-1.0)

    def emit_norm_pair(qc, hp):
        # per-pair broadcast of 1/denominator and normalize into ctxT
        qs = slice(qc * 512, (qc + 1) * 512)
        for hp in (hp,):
            bcE = ps_m.tile([HD, 512], F32, tag="mm512")
            bcO = ps_m.tile([P, 512], F32, tag="mm512")
            nc.tensor.matmul(
                bcE[:],
                sel_sb[64:68, 64 * hp : 64 * hp + HD],
                rec_sb[64:68, qs],
                start=True,
                stop=True,
            )
            nc.tensor.matmul(
                bcO[HD:P, :],
                sel_sb[0:4, 256 + 64 * hp : 256 + 64 * hp + HD],
                rec_sb[0:4, qs],
                start=True,
                stop=True,
            )
            bc_sb = bc_pool.tile([P, 512], BF16)
            nc.vector.tensor_copy(bc_sb[0:HD, :], bcE[:])
            nc.vector.tensor_copy(bc_sb[HD:P, :], bcO[HD:P, :])
            nc.vector.tensor_mul(ctxT_sb[:, hp, qs], uctx_sb[:, hp, qs], bc_sb[:])

    def emit_outproj_tt(tt):
        # out[t, o] = sum_f ctxT[f, t] * woT[f, o] for one 128-token tile
        out_t = o_pool.tile([P, D], F32)
        for half in range(2):
            acc = ps_m.tile([P, 512], F32, tag="mm512")
            for ft in range(4):
                nc.tensor.matmul(
                    acc[:],
                    ctxT_sb[:, ft, tt * P : (tt + 1) * P],
                    wo_sb[:, ft, half * 512 : (half + 1) * 512],
                    start=(ft == 0),
                    stop=(ft == 3),
                )
            nc.vector.tensor_copy(out_t[:, half * 512 : (half + 1) * 512], acc[:])
            nc.sync.dma_start(
                out[tt * P : (tt + 1) * P, half * 512 : (half + 1) * 512],
                out_t[:, half * 512 : (half + 1) * 512],
            )

    def emit_qkproj_piece(qc, ft):
        # Q and K projections for feature-tile ft of chunk qc
        qs = slice(qc * 512, (qc + 1) * 512)
        for w_sb, dst in ((wq_sb, qt_sb), (wk_sb, kt_sb)):
            acc = ps_m.tile([P, 512], F32, tag="mm512")
            for kt in range(NKT):
                nc.tensor.matmul(
                    acc[:],
                    w_sb[:, kt, ft * P : (ft + 1) * P],
                    xT_sb[:, kt, qs],
                    start=(kt == 0),
                    stop=(kt == NKT - 1),
                )
            nc.vector.tensor_copy(dst[:, ft, qs], acc[:])

    def emit_vproj_piece(tt):
        # V projection (token-major) for one 128-token key tile
        acc = ps_m.tile([P, 512], F32, tag="mm512")
        for kt in range(NKT):
            nc.tensor.matmul(
                acc[:],
                xT_sb[:, kt, tt * P : (tt + 1) * P],
                wv_sb[:, kt, :],
                start=(kt == 0),
                stop=(kt == NKT - 1),
            )
        # scatter per-head 64-col blocks into the V_ext layout
        nc.vector.tensor_copy(
            v_sb[:, tt, 0:OBASE].rearrange("p (j c) -> p j c", j=4)[:, :, 0:HD],
            acc[:].rearrange("p (j c) -> p j c", j=4)[:, :, 0:HD],
        )
        nc.vector.tensor_copy(
            v_sb[:, tt, OBASE:VROW].rearrange("p (j c) -> p j c", j=4)[
                :, :, HD : 2 * HD
            ],
            acc[:].rearrange("p (j c) -> p j c", j=4)[:, :, HD : 2 * HD],
        )

    def emit_proj(qc):
        for ft in range(4):
            emit_qkproj_piece(qc, ft)
        for tt in range(4 * qc, 4 * qc + 4):
            emit_vproj_piece(tt)

    # ---- pipelined main loop: chunk qc's attention interleaves the previous
    # chunk's normalization + output projection and the NEXT chunk's
    # projections, keeping the tensor engine dense while ACT paces softmax ----
    emit_proj(0)
    for qc in range(NQC):
        qs = slice(qc * 512, (qc + 1) * 512)
        nc.vector.memset(den_sb[64:68, qs], 0.0)
        nc.vector.memset(den_sb[0:4, qs], 0.0)
        n_kt = 4 * qc + 4
        for hp in range(4):
            if qc >= 1:
                emit_outproj_tt(4 * (qc - 1) + hp)
            if qc < NQC - 1:
                emit_qkproj_piece(qc + 1, hp)
                emit_vproj_piece(4 * (qc + 1) + hp)
            ctxE = ps_c.tile([72, 512], F32, tag="ctx")
            ctxO = ps_c.tile([P, 512], F32, tag="ctx")
            for kt in range(n_kt):
                j = kt - 4 * qc
                lo = 128 * j if j > 0 else 0
                # diagonal tiles only need query columns >= lo; the skipped
                # region is never read downstream (ctx matmuls are range-
                # restricted the same way), so scores/exp shrink with it
                sc = ps_s.tile([P, 1024], F32)
                nc.tensor.matmul(
                    sc[:, lo:512],
                    kt_sb[0:HD, hp, kt * P : (kt + 1) * P],
                    qt_sb[0:HD, hp, qc * 512 + lo : (qc + 1) * 512],
                    start=True,
                    stop=True,
                )
                nc.tensor.matmul(
                    sc[:, 512 + lo : 1024],
                    kt_sb[HD:P, hp, kt * P : (kt + 1) * P],
                    qt_sb[HD:P, hp, qc * 512 + lo : (qc + 1) * 512],
                    start=True,
                    stop=True,
                )
                pt = p_pool.tile([P, 1024], BF16)
                nc.scalar.activation(pt[:, lo:1024], sc[:, lo:1024], Exp, scale=0.125)
                if j >= 0:
                    nc.vector.tensor_mul(
                        pt[:, 128 * j : 128 * (j + 1)],
                        pt[:, 128 * j : 128 * (j + 1)],
                        tri_sb[:],
                    )
                    nc.vector.tensor_mul(
                        pt[:, 512 + 128 * j : 512 + 128 * (j + 1)],
                        pt[:, 512 + 128 * j : 512 + 128 * (j + 1)],
                        tri_sb[:],
                    )
                nc.tensor.matmul(
                    ctxE[:, lo:512],
                    v_sb[:, kt, ESTR * hp : ESTR * hp + 72],
                    pt[:, lo:512],
                    start=(kt == 0),
                    stop=(kt == n_kt - 1),
                )
                nc.tensor.matmul(
                    ctxO[:, lo:512],
                    v_sb[:, kt, OBASE + 128 * hp : OBASE + 128 * (hp + 1)],
                    pt[:, 512 + lo : 1024],
                    start=(kt == 0),
                    stop=(kt == n_kt - 1),
                )
            # denominators: pair hp's denom sits in row 64+hp (even) / hp
            # (odd); other rows of the 4-row aligned block are zero, so a
            # 4-row add merges all pairs without unaligned partition starts.
            nc.vector.tensor_add(den_sb[64:68, qs], den_sb[64:68, qs], ctxE[64:68, :])
            nc.vector.tensor_add(den_sb[0:4, qs], den_sb[0:4, qs], ctxO[0:4, :])
            nc.vector.tensor_copy(uctx_sb[0:HD, hp, qs], ctxE[0:HD, :])
            nc.vector.tensor_copy(uctx_sb[HD:P, hp, qs], ctxO[HD:P, :])
            # normalize pairs as soon as their denominators are final; the
            # second recip redundantly recovers rows of the first (the 4-row
            # block keeps partition starts 32-aligned) which is idempotent
            if hp == 1:
                emit_recip(qc, 2)
                emit_norm_pair(qc, 0)
                emit_norm_pair(qc, 1)
            elif hp == 3:
                emit_recip(qc, 4)
                emit_norm_pair(qc, 2)
                emit_norm_pair(qc, 3)

    for tt in range(4 * (NQC - 1), NTT):
        emit_outproj_tt(tt)


def _get_nc():
    if "nc" not in _NC_CACHE:
        _NC_CACHE["nc"] = _build_nc()
    return _NC_CACHE["nc"]


def _prep_inputs(x, W_q, W_k, W_v, W_o):
    bf = ml_dtypes.bfloat16
    tri = np.triu(np.ones((P, P), np.float32)).astype(bf)
    sel = np.zeros((P, 2 * 4 * HD), np.float32)
    for j in range(4):
        sel[64 + j, 64 * j : 64 * j + HD] = 1.0   # even-head selector block j
        sel[j, 256 + 64 * j : 256 + 64 * j + HD] = 1.0  # odd-head selector
    in_maps = []
    for c in range(8):
        b, hg = c // 2, c % 2
        rows = slice(hg * DH, (hg + 1) * DH)
        in_maps.append(
            {
                "xT": np.ascontiguousarray(x[b].T).astype(bf),
                "wqT": np.ascontiguousarray(W_q[rows, :].T).astype(bf),
                "wkT": np.ascontiguousarray(W_k[rows, :].T).astype(bf),
                "wvT": np.ascontiguousarray(W_v[rows, :].T).astype(bf),
                "woT": np.ascontiguousarray(W_o[:, rows].T).astype(bf),
                "tri": tri,
                "sel": sel,
            }
        )
    return in_maps


def kernel(x, W_q, W_k, W_v, W_o, _spmd_kwargs=None):
    x = np.asarray(x, dtype=np.float32)
    W_q = np.asarray(W_q, dtype=np.float32)
    W_k = np.asarray(W_k, dtype=np.float32)
    W_v = np.asarray(W_v, dtype=np.float32)
    W_o = np.asarray(W_o, dtype=np.float32)

    nc = _get_nc()
    in_maps = _prep_inputs(x, W_q, W_k, W_v, W_o)
    res = run_bass_kernel_spmd(
        nc, in_maps, core_ids=list(range(8)), **(_spmd_kwargs or {})
    )
    out = np.empty((B, S, D), np.float32)
    for b in range(B):
        out[b] = res.results[2 * b]["out"] + res.results[2 * b + 1]["out"]
    if _spmd_kwargs:
        kernel.last_results = res
    return out


# revision 33
# speedup vs baseline: 1.0328x; 1.0328x over previous
"""Multi-head causal attention (B=4, S=2048, D=1024, H=16, HD=64) on 8 trn2 cores.

Sharding: core c = (batch b = c//2, head-group hg = c%2, 8 heads each).
Each core computes Q/K/V projections restricted to its head-group, causal
attention for those heads, and a partial output projection (contraction over
its 512 ctx features). Host sums the two partial outputs per batch.

Per-core kernel layout choices:
  - QT/KT stored feature-major [head_dim, tokens] with head pairs stacked on
    partitions (0-63 / 64-127) so score matmuls for a pair run concurrently
    via PE row tiling.
  - Scores are computed transposed (ST[k, q] = K_h @ Q_h^T) so the exp'd P
    tiles are already key-major, exactly what the ctx matmul (lhsT=V) needs.
  - Softmax denominators come free out of the ctx matmul via one-hot columns
    appended to V (even heads: feat rows 0..63, denom row 64+pair; odd heads:
    feat rows 64..127, denom row pair — every evacuation stays on its own
    partitions, no cross-partition moves anywhere).
  - Normalization: reciprocal of denominators, broadcast across partitions
    with a tiny selector matmul, then one multiply per head pair.
"""

from contextlib import ExitStack

import numpy as np
import ml_dtypes

import concourse.tile as tile
from concourse import bacc, mybir
from concourse.bass_utils import run_bass_kernel_spmd

BF16 = mybir.dt.bfloat16
F32 = mybir.dt.float32

B, S, D = 4, 2048, 1024
HG = 8            # heads per core
HD = 64           # head dim
DH = HG * HD      # 512 features per core
P = 128
NKT = D // P      # 8 contraction tiles over model dim
NTT = S // P      # 16 token tiles of 128
NQC = S // 512    # 4 query chunks of 512
ESTR = 74         # even-head V block stride: 64 feat + 8 one-hot + 2 pad
OBASE = 4 * ESTR  # 296: start of odd-head V blocks (128 cols each)
VROW = OBASE + 4 * 128  # 808 cols per key-tile row of V_sb

_NC_CACHE = {}


def _build_nc():
    nc = bacc.Bacc("TRN2", target_bir_lowering=False, debug=False)

    xT = nc.dram_tensor("xT", [D, S], BF16, kind="ExternalInput")
    wqT = nc.dram_tensor("wqT", [D, DH], BF16, kind="ExternalInput")
    wkT = nc.dram_tensor("wkT", [D, DH], BF16, kind="ExternalInput")
    wvT = nc.dram_tensor("wvT", [D, DH], BF16, kind="ExternalInput")
    woT = nc.dram_tensor("woT", [DH, D], BF16, kind="ExternalInput")
    tri = nc.dram_tensor("tri", [P, P], BF16, kind="ExternalInput")
    sel = nc.dram_tensor("sel", [P, 2 * 4 * HD], F32, kind="ExternalInput")
    out = nc.dram_tensor("out", [S, D], F32, kind="ExternalOutput")

    with tile.TileContext(nc) as tc, ExitStack() as ctx:
        _emit(ctx, tc, nc, xT, wqT, wkT, wvT, woT, tri, sel, out)
    nc.compile()
    return nc


def _emit(ctx, tc, nc, xT, wqT, wkT, wvT, woT, tri, sel, out):
    Exp = mybir.ActivationFunctionType.Exp

    sb = ctx.enter_context(tc.tile_pool(name="sb", bufs=1))
    p_pool = ctx.enter_context(tc.tile_pool(name="p", bufs=6))
    bc_pool = ctx.enter_context(tc.tile_pool(name="bc", bufs=2))
    o_pool = ctx.enter_context(tc.tile_pool(name="o", bufs=2))
    ps_s = ctx.enter_context(tc.tile_pool(name="ps_s", bufs=2, space="PSUM"))
    ps_c = ctx.enter_context(tc.tile_pool(name="ps_c", bufs=2, space="PSUM"))
    ps_m = ctx.enter_context(tc.tile_pool(name="ps_m", bufs=2, space="PSUM"))

    # ---- persistent SBUF tensors ----
    xT_sb = sb.tile([P, NKT, S], BF16)
    wq_sb = sb.tile([P, NKT, DH], BF16)
    wk_sb = sb.tile([P, NKT, DH], BF16)
    wv_sb = sb.tile([P, NKT, DH], BF16)
    wo_sb = sb.tile([P, DH // P, D], BF16)
    tri_sb = sb.tile([P, P], BF16)
    qt_sb = sb.tile([P, 4, S], BF16)      # [hd (2 heads), pair, tokens]
    kt_sb = sb.tile([P, 4, S], BF16)
    v_sb = sb.tile([P, NTT, VROW], BF16)
    uctx_sb = sb.tile([P, 4, S], BF16)    # unnormalized ctxT, pair-stacked
    ctxT_sb = sb.tile([P, 4, S], BF16)    # normalized ctxT, pair-stacked
    den_sb = sb.tile([P, S], BF16)        # rows 0-3 (odd heads), 64-67 (even)
    rec_sb = sb.tile([P, S], F32)
    scr_sb = sb.tile([P, 512], F32)       # ln(den) scratch for the ACT recip
    sel_sb = sb.tile([P, 2 * 4 * HD], F32)

    # DMA order: what the first projection needs comes first
    xT_r = xT.rearrange("(t p) n -> p t n", p=P)
    wq_r = wqT.rearrange("(t p) n -> p t n", p=P)
    nc.sync.dma_start(xT_sb[:, :, 0:512], xT_r[:, :, 0:512])
    for ft in range(4):
        # per-feature-tile chunks so Q-proj(ft=0) starts as early as possible
        nc.gpsimd.dma_start(
            wq_sb[:, :, ft * P : (ft + 1) * P], wq_r[:, :, ft * P : (ft + 1) * P]
        )
    nc.gpsimd.dma_start(wk_sb[:], wkT.rearrange("(t p) n -> p t n", p=P))
    nc.sync.dma_start(wv_sb[:], wvT.rearrange("(t p) n -> p t n", p=P))
    nc.sync.dma_start(tri_sb[:], tri[:])
    for qc in range(1, NQC):
        nc.sync.dma_start(
            xT_sb[:, :, qc * 512 : (qc + 1) * 512], xT_r[:, :, qc * 512 : (qc + 1) * 512]
        )
    nc.gpsimd.dma_start(wo_sb[:], woT.rearrange("(t p) n -> p t n", p=P))
    nc.gpsimd.dma_start(sel_sb[:], sel[:])

    # ---- constant patterns: V one-hot columns ----
    nc.vector.memset(v_sb[:], 0.0)
    # rec rows read by the selector matmul before first written: keep finite
    nc.vector.memset(rec_sb[64:68, :], 1.0)
    nc.vector.memset(rec_sb[0:4, :], 1.0)
    for j in range(4):
        # even head 2j: one at col 74j + 64 + j  (-> denom row 64+j)
        nc.vector.memset(v_sb[:, :, 75 * j + 64 : 75 * j + 65], 1.0)
        # odd head 2j+1: one at col 296 + 128j + j  (-> denom row j)
        nc.vector.memset(v_sb[:, :, OBASE + 129 * j : OBASE + 129 * j + 1], 1.0)

    def emit_recip(qc, rows):
        qs = slice(qc * 512, (qc + 1) * 512)
        nc.vector.reciprocal(rec_sb[64 : 64 + rows, qs], den_sb[64 : 64 + rows, qs])
        nc.vector.reciprocal(rec_sb[0:rows, qs], den_sb[0:rows, qs])

    def emit_norm_pair(qc, hp):
        # per-pair broadcast of 1/denominator and normalize into ctxT
        qs = slice(qc * 512, (qc + 1) * 512)
        for hp in (hp,):
            bcE = ps_m.tile([HD, 512], F32, tag="mm512")
            bcO = ps_m.tile([P, 512], F32, tag="mm512")
            nc.tensor.matmul(
                bcE[:],
                sel_sb[64:68, 64 * hp : 64 * hp + HD],
                rec_sb[64:68, qs],
                start=True,
                stop=True,
            )
            nc.tensor.matmul(
                bcO[HD:P, :],
                sel_sb[0:4, 256 + 64 * hp : 256 + 64 * hp + HD],
                rec_sb[0:4, qs],
                start=True,
                stop=True,
            )
            bc_sb = bc_pool.tile([P, 512], BF16)
            nc.vector.tensor_copy(bc_sb[0:HD, :], bcE[:])
            nc.vector.tensor_copy(bc_sb[HD:P, :], bcO[HD:P, :])
            nc.vector.tensor_mul(ctxT_sb[:, hp, qs], uctx_sb[:, hp, qs], bc_sb[:])

    def emit_outproj_tt(tt):
        # out[t, o] = sum_f ctxT[f, t] * woT[f, o] for one 128-token tile
        out_t = o_pool.tile([P, D], F32)
        for half in range(2):
            acc = ps_m.tile([P, 512], F32, tag="mm512")
            for ft in range(4):
                nc.tensor.matmul(
                    acc[:],
                    ctxT_sb[:, ft, tt * P : (tt + 1) * P],
                    wo_sb[:, ft, half * 512 : (half + 1) * 512],
                    start=(ft == 0),
                    stop=(ft == 3),
                )
            nc.vector.tensor_copy(out_t[:, half * 512 : (half + 1) * 512], acc[:])
            nc.sync.dma_start(
                out[tt * P : (tt + 1) * P, half * 512 : (half + 1) * 512],
                out_t[:, half * 512 : (half + 1) * 512],
            )

    def emit_qkproj_piece(qc, ft):
        # Q and K projections for feature-tile ft of chunk qc
        qs = slice(qc * 512, (qc + 1) * 512)
        for w_sb, dst in ((wq_sb, qt_sb), (wk_sb, kt_sb)):
            acc = ps_m.tile([P, 512], F32, tag="mm512")
            for kt in range(NKT):
                nc.tensor.matmul(
                    acc[:],
                    w_sb[:, kt, ft * P : (ft + 1) * P],
                    xT_sb[:, kt, qs],
                    start=(kt == 0),
                    stop=(kt == NKT - 1),
                )
            nc.vector.tensor_copy(dst[:, ft, qs], acc[:])

    def emit_vproj_piece(tt):
        # V projection (token-major) for one 128-token key tile
        acc = ps_m.tile([P, 512], F32, tag="mm512")
        for kt in range(NKT):
            nc.tensor.matmul(
                acc[:],
                xT_sb[:, kt, tt * P : (tt + 1) * P],
                wv_sb[:, kt, :],
                start=(kt == 0),
                stop=(kt == NKT - 1),
            )
        # scatter per-head 64-col blocks into the V_ext layout
        nc.vector.tensor_copy(
            v_sb[:, tt, 0:OBASE].rearrange("p (j c) -> p j c", j=4)[:, :, 0:HD],
            acc[:].rearrange("p (j c) -> p j c", j=4)[:, :, 0:HD],
        )
        nc.vector.tensor_copy(
            v_sb[:, tt, OBASE:VROW].rearrange("p (j c) -> p j c", j=4)[
                :, :, HD : 2 * HD
            ],
            acc[:].rearrange("p (j c) -> p j c", j=4)[:, :, HD : 2 * HD],
        )

    def emit_proj(qc):
        for ft in range(4):
            emit_qkproj_piece(qc, ft)
        for tt in range(4 * qc, 4 * qc + 4):
            emit_vproj_piece(tt)

    # ---- pipelined main loop: chunk qc's attention interleaves the previous
    # chunk's normalization + output projection and the NEXT chunk's
    # projections, keeping the tensor engine dense while ACT paces softmax ----
    emit_proj(0)
    for qc in range(NQC):
        qs = slice(qc * 512, (qc + 1) * 512)
        nc.vector.memset(den_sb[64:68, qs], 0.0)
        nc.vector.memset(den_sb[0:4, qs], 0.0)
        n_kt = 4 * qc + 4
        for hp in range(4):
            if qc < NQC - 1:
                emit_qkproj_piece(qc + 1, hp)
                emit_vproj_piece(4 * (qc + 1) + hp)
            if qc >= 1:
                emit_outproj_tt(4 * (qc - 1) + hp)
            ctxE = ps_c.tile([72, 512], F32, tag="ctx")
            ctxO = ps_c.tile([P, 512], F32, tag="ctx")
            for kt in range(n_kt):
                j = kt - 4 * qc
                lo = 128 * j if j > 0 else 0
                # diagonal tiles only need query columns >= lo; the skipped
                # region is never read downstream (ctx matmuls are range-
                # restricted the same way), so scores/exp shrink with it
                sc = ps_s.tile([P, 1024], F32)
                nc.tensor.matmul(
                    sc[:, lo:512],
                    kt_sb[0:HD, hp, kt * P : (kt + 1) * P],
                    qt_sb[0:HD, hp, qc * 512 + lo : (qc + 1) * 512],
                    start=True,
                    stop=True,
                )
                nc.tensor.matmul(
                    sc[:, 512 + lo : 1024],
                    kt_sb[HD:P, hp, kt * P : (kt + 1) * P],
                    qt_sb[HD:P, hp, qc * 512 + lo : (qc + 1) * 512],
                    start=True,
                    stop=True,
                )
                pt = p_pool.tile([P, 1024], BF16)
                nc.scalar.activation(pt[:, lo:1024], sc[:, lo:1024], Exp, scale=0.125)
                if j >= 0:
                    nc.vector.tensor_mul(
                        pt[:, 128 * j : 128 * (j + 1)],
                        pt[:, 128 * j : 128 * (j + 1)],
                        tri_sb[:],
                    )
                    nc.vector.tensor_mul(
                        pt[:, 512 + 128 * j : 512 + 128 * (j + 1)],
                        pt[:, 512 + 128 * j : 512 + 128 * (j + 1)],
                        tri_sb[:],
                    )
                nc.tensor.matmul(
                    ctxE[:, lo:512],
                    v_sb[:, kt, ESTR * hp : ESTR * hp + 72],
                    pt[:, lo:512],
                    start=(kt == 0),
                    stop=(kt == n_kt - 1),
                )
                nc.tensor.matmul(
                    ctxO[:, lo:512],
                    v_sb[:, kt, OBASE + 128 * hp : OBASE + 128 * (hp + 1)],
                    pt[:, 512 + lo : 1024],
                    start=(kt == 0),
                    stop=(kt == n_kt - 1),
                )
            # denominators: pair hp's denom sits in row 64+hp (even) / hp
            # (odd); other rows of the 4-row aligned block are zero, so a
            # 4-row add merges all pairs without unaligned partition starts.
            nc.vector.tensor_add(den_sb[64:68, qs], den_sb[64:68, qs], ctxE[64:68, :])
            nc.vector.tensor_add(den_sb[0:4, qs], den_sb[0:4, qs], ctxO[0:4, :])
            nc.vector.tensor_copy(uctx_sb[0:HD, hp, qs], ctxE[0:HD, :])
            nc.vector.tensor_copy(uctx_sb[HD:P, hp, qs], ctxO[HD:P, :])
            # normalize once all pairs' denominators are final; consumers
            # (outproj) are deferred into the next chunk's sections, so the
            # reciprocal latency hides behind attention/projection work
            if hp == 3:
                emit_recip(qc, 4)
                for p_ in range(4):
                    emit_norm_pair(qc, p_)

    for tt in range(4 * (NQC - 1), NTT):
        emit_outproj_tt(tt)


def _get_nc():
    if "nc" not in _NC_CACHE:
        _NC_CACHE["nc"] = _build_nc()
    return _NC_CACHE["nc"]


def _prep_inputs(x, W_q, W_k, W_v, W_o):
    bf = ml_dtypes.bfloat16
    tri = np.triu(np.ones((P, P), np.float32)).astype(bf)
    sel = np.zeros((P, 2 * 4 * HD), np.float32)
    for j in range(4):
        sel[64 + j, 64 * j : 64 * j + HD] = 1.0   # even-head selector block j
        sel[j, 256 + 64 * j : 256 + 64 * j + HD] = 1.0  # odd-head selector
    in_maps = []
    for c in range(8):
        b, hg = c // 2, c % 2
        rows = slice(hg * DH, (hg + 1) * DH)
        in_maps.append(
            {
                "xT": np.ascontiguousarray(x[b].T).astype(bf),
                "wqT": np.ascontiguousarray(W_q[rows, :].T).astype(bf),
                "wkT": np.ascontiguousarray(W_k[rows, :].T).astype(bf),
                "wvT": np.ascontiguousarray(W_v[rows, :].T).astype(bf),
                "woT": np.ascontiguousarray(W_o[:, rows].T).astype(bf),
                "tri": tri,
                "sel": sel,
            }
        )
    return in_maps


def kernel(x, W_q, W_k, W_v, W_o, _spmd_kwargs=None):
    x = np.asarray(x, dtype=np.float32)
    W_q = np.asarray(W_q, dtype=np.float32)
    W_k = np.asarray(W_k, dtype=np.float32)
    W_v = np.asarray(W_v, dtype=np.float32)
    W_o = np.asarray(W_o, dtype=np.float32)

    nc = _get_nc()
    in_maps = _prep_inputs(x, W_q, W_k, W_v, W_o)
    res = run_bass_kernel_spmd(
        nc, in_maps, core_ids=list(range(8)), **(_spmd_kwargs or {})
    )
    out = np.empty((B, S, D), np.float32)
    for b in range(B):
        out[b] = res.results[2 * b]["out"] + res.results[2 * b + 1]["out"]
    if _spmd_kwargs:
        kernel.last_results = res
    return out
